# revision 10
# baseline (speedup 1.0000x reference)
"""Trainium2 Bass kernel for the LossLinkerE2E problem.

Computes, for each batch element b (one NeuronCore per batch element, B=8):
    candidates/lengths/targets/linker_vecs gathered along the span axis by
    linker_indices; cand_vecs = ent_table[candidates]; a fused 2-layer MLP
    score; masked BCE-with-logits loss (summed) and argmax predictions.

Host-side input prep (layout only): W2 is folded into W1/b1 columns
(u_h = W2_h * h_h) and the H axis is permuted so positive-W2 columns come
first.  Then scores = sum_pos relu(u) + sum_neg min(u, 0), which lets the
final H-reduction run on the Vector/Scalar engines in true fp32 (the PE's
fp32 matmul path truncates operands to ~FP22, which is too coarse for the
argmax: candidate scores within a span differ only by the tiny h_cand
contribution).

Per-core pipeline:
  - indirect-DMA gathers (linker rows, candidate ids, targets, lengths,
    entity rows), PE transposes to get [E, spans] / [D, spans] layouts,
  - span-term g = linker_vecs @ W1s' (+b1') computed once per span block,
  - per candidate: h = g (re-injected via an identity matmul, common-mode
    exact) + cand_vecs @ W1e' accumulated in PSUM in float32r,
  - fused tensor_tensor_reduce (relu/min + add-accumulate) for scores,
  - small-vector ops for mask, BCE, loss partials and argmax.
"""

import numpy as np
from contextlib import ExitStack

B, S, C = 8, 512, 16
N, D, E, H, V = 4096, 1024, 256, 1024, 100000
P = 128
SB = S // P      # span blocks per core
HC = H // 512    # H chunks
EC = E // P      # E chunks
DC = D // P      # D chunks
NCORES = 8


def _build_program(Pt):
    """Build the Bass/Tile program. Pt[hc] = pos/neg boundary inside each
    512-wide H chunk (compile-time constants derived from W2's signs)."""
    import concourse.bass as bass
    import concourse.bacc as bacc
    import concourse.tile as tile
    import concourse.mybir as mybir

    dt = mybir.dt
    Alu = mybir.AluOpType
    Act = mybir.ActivationFunctionType
    f32, f32r, i32 = dt.float32, dt.float32r, dt.int32

    nc = bacc.Bacc("TRN2", debug=False, enable_asserts=False)

    sv_d = nc.dram_tensor("span_vecs", [N, D], f32, kind="ExternalInput").ap()
    ent_d = nc.dram_tensor("ent_table", [V, E], f32, kind="ExternalInput").ap()
    w1s_d = nc.dram_tensor("w1s", [D, H], f32r, kind="ExternalInput").ap()
    w1e_d = nc.dram_tensor("w1e", [E, H], f32r, kind="ExternalInput").ap()
    b1r_d = nc.dram_tensor("b1rep", [P, H], f32, kind="ExternalInput").ap()
    tgt_d = nc.dram_tensor("targets", [N, C], f32, kind="ExternalInput").ap()
    lidx_d = nc.dram_tensor("lidx", [P, SB], i32, kind="ExternalInput").ap()
    cnd_d = nc.dram_tensor("cands", [N, C], i32, kind="ExternalInput").ap()
    len_d = nc.dram_tensor("lens", [N, 1], i32, kind="ExternalInput").ap()
    id_d = nc.dram_tensor("ident", [P, P], f32, kind="ExternalInput").ap()
    idr_d = nc.dram_tensor("identr", [P, P], f32r, kind="ExternalInput").ap()
    io_d = nc.dram_tensor("iota", [P, C], f32, kind="ExternalInput").ap()
    iom_d = nc.dram_tensor("iotam", [P, C], f32, kind="ExternalInput").ap()
    b2_d = nc.dram_tensor("b2c", [P, 1], f32, kind="ExternalInput").ap()

    sco_d = nc.dram_tensor("scores_o", [S, C], f32, kind="ExternalOutput").ap()
    prd_d = nc.dram_tensor("preds_o", [S, 1], i32, kind="ExternalOutput").ap()
    los_d = nc.dram_tensor("loss_o", [P, 1], f32, kind="ExternalOutput").ap()

    with tile.TileContext(nc) as tc, ExitStack() as ctx:
        const = ctx.enter_context(tc.tile_pool(name="const", bufs=1))
        tp_ps = ctx.enter_context(tc.tile_pool(name="tp", bufs=2, space="PSUM"))
        g_ps = ctx.enter_context(tc.tile_pool(name="gp", bufs=2, space="PSUM"))
        h_ps = ctx.enter_context(tc.tile_pool(name="hp", bufs=4, space="PSUM"))
        candp = ctx.enter_context(tc.tile_pool(name="cand", bufs=2))
        ctp = ctx.enter_context(tc.tile_pool(name="candT", bufs=2))
        scrp = ctx.enter_context(tc.tile_pool(name="scr", bufs=2))
        outp = ctx.enter_context(tc.tile_pool(name="outp", bufs=2))

        # ---- constants / weights ----
        w1s = const.tile([P, DC, H], f32r)
        nc.sync.dma_start(w1s[:], w1s_d.rearrange("(c p) h -> p c h", p=P))
        w1e = const.tile([P, EC, H], f32r)
        nc.sync.dma_start(w1e[:], w1e_d.rearrange("(c p) h -> p c h", p=P))
        b1r = const.tile([P, H], f32)
        nc.sync.dma_start(b1r[:], b1r_d[:])
        ident = const.tile([P, P], f32)
        nc.sync.dma_start(ident[:], id_d[:])
        identr = const.tile([P, P], f32r)
        nc.sync.dma_start(identr[:], idr_d[:])
        iota = const.tile([P, C], f32)
        nc.sync.dma_start(iota[:], io_d[:])
        iotam = const.tile([P, C], f32)
        nc.sync.dma_start(iotam[:], iom_d[:])
        b2 = const.tile([P, 1], f32)
        nc.sync.dma_start(b2[:], b2_d[:])
        lidx = const.tile([P, SB], i32)
        nc.sync.dma_start(lidx[:], lidx_d[:])
        zero1 = const.tile([P, 1], f32)
        nc.vector.memset(zero1[:], 0.0)

        # ---- span-axis gathers (one offset column per indirect DMA: the HW
        # descriptor expansion only honours a single offset per partition) ----
        lv = const.tile([P, SB, D], f32)
        cidx = const.tile([P, SB, C], i32)
        tgt = const.tile([P, SB, C], f32)
        leni = const.tile([P, SB, 1], i32)
        for n in range(SB):
            off = bass.IndirectOffsetOnAxis(ap=lidx[:, n:n + 1], axis=0)
            nc.gpsimd.indirect_dma_start(out=lv[:, n, :], out_offset=None,
                                         in_=sv_d[:], in_offset=off)
            nc.gpsimd.indirect_dma_start(out=cidx[:, n, :], out_offset=None,
                                         in_=cnd_d[:], in_offset=off)
            nc.gpsimd.indirect_dma_start(out=tgt[:, n, :], out_offset=None,
                                         in_=tgt_d[:], in_offset=off)
            nc.gpsimd.indirect_dma_start(out=leni[:, n, :], out_offset=None,
                                         in_=len_d[:], in_offset=off)

        # ---- linkerT = lv^T  ([D, spans], 8 chunks of [128, 512]) ----
        lt = const.tile([P, DC, S], f32r)
        for d in range(DC):
            pt = tp_ps.tile([P, S], f32, tag="tp")
            for n in range(SB):
                nc.tensor.transpose(pt[:, bass.ts(n, P)],
                                    lv[:, n, bass.ts(d, P)], ident[:])
            nc.scalar.copy(lt[:, d, :], pt[:])

        # ---- span term g = lv @ W1s' + b1'  ([spans, H] per (block, hc)) ----
        g_all = const.tile([P, SB * H], f32r)
        for n in range(SB):
            for hc in range(HC):
                gp = g_ps.tile([P, 512], f32, tag="g")
                for d in range(DC):
                    nc.tensor.matmul(
                        gp[:],
                        lhsT=lt[:, d, bass.ts(n, P)],
                        rhs=w1s[:, d, bass.ts(hc, 512)],
                        start=(d == 0), stop=(d == DC - 1))
                # copy out + bias add in one pass (fp32-exact)
                nc.vector.scalar_tensor_tensor(
                    out=g_all[:, bass.ts(n * HC + hc, 512)],
                    in0=gp[:], scalar=0.0, in1=b1r[:, bass.ts(hc, 512)],
                    op0=Alu.bypass, op1=Alu.add)

        # ---- masks ----
        lenf = const.tile([P, SB], f32)
        nc.vector.tensor_copy(lenf[:], leni[:, :, 0])
        mask = const.tile([P, SB, C], f32)
        m2 = const.tile([P, SB, C], f32)
        for n in range(SB):
            nc.vector.tensor_scalar(out=mask[:, n, :], in0=iota[:],
                                    scalar1=lenf[:, n:n + 1], scalar2=None,
                                    op0=Alu.is_lt)
            nc.vector.tensor_scalar(out=m2[:, n, :], in0=mask[:, n, :],
                                    scalar1=1.0, scalar2=1e23,
                                    op0=Alu.subtract, op1=Alu.mult)

        # ---- per-candidate main loop ----
        red = const.tile([P, SB, C, 2 * HC], f32)
        nc.vector.memset(red[:], 0.0)

        for c in range(C):
            cv = candp.tile([P, SB, E], f32, tag="cv")
            for n in range(SB):
                nc.gpsimd.indirect_dma_start(
                    out=cv[:, n, :], out_offset=None, in_=ent_d[:],
                    in_offset=bass.IndirectOffsetOnAxis(ap=cidx[:, n, c:c + 1],
                                                        axis=0))
            ct = []
            for e in range(EC):
                ptile = tp_ps.tile([P, S], f32, tag="tp")
                for n in range(SB):
                    nc.tensor.transpose(ptile[:, bass.ts(n, P)],
                                        cv[:, n, bass.ts(e, P)], ident[:])
                cte = ctp.tile([P, S], f32r, tag=f"ct{e}")
                nc.scalar.copy(cte[:], ptile[:])
                ct.append(cte)

            for n in range(SB):
                for hc in range(HC):
                    h = h_ps.tile([P, 512], f32, tag="h")
                    nc.tensor.matmul(
                        h[:], lhsT=identr[:],
                        rhs=g_all[:, bass.ts(n * HC + hc, 512)],
                        start=True, stop=False)
                    for e in range(EC):
                        nc.tensor.matmul(
                            h[:], lhsT=ct[e][:, bass.ts(n, P)],
                            rhs=w1e[:, e, bass.ts(hc, 512)],
                            start=False, stop=(e == EC - 1))
                    p_ = Pt[hc]
                    if p_ > 0:
                        s1 = scrp.tile([P, 512], f32, tag="scra")
                        nc.scalar.activation(
                            s1[:, :p_], h[:, :p_], Act.Relu,
                            accum_out=red[:, n, c, 2 * hc:2 * hc + 1])
                    if p_ < 512:
                        s2 = scrp.tile([P, 512], f32, tag="scr")
                        nc.vector.scalar_tensor_tensor(
                            out=s2[:, :512 - p_], in0=h[:, p_:], scalar=0.0,
                            in1=zero1[:].to_broadcast([P, 512 - p_]),
                            op0=Alu.min, op1=Alu.add,
                            accum_out=red[:, n, c, 2 * hc + 1:2 * hc + 2])

        # ---- scores, mask/argmax, bce/loss ----
        losscol = outp.tile([P, SB], f32, bufs=1)
        for n in range(SB):
            sc = outp.tile([P, C], f32, tag="sc")
            nc.vector.tensor_reduce(out=sc[:], in_=red[:, n, :, :],
                                    axis=mybir.AxisListType.X, op=Alu.add)
            sc2 = outp.tile([P, C], f32, tag="sc2")
            nc.vector.tensor_scalar(out=sc2[:], in0=sc[:],
                                    scalar1=b2[:, 0:1], scalar2=None,
                                    op0=Alu.add)
            nc.sync.dma_start(sco_d[bass.ts(n, P), :], sc2[:])

            mskd = outp.tile([P, C], f32, tag="mskd")
            nc.vector.tensor_add(mskd[:], sc2[:], m2[:, n, :])
            rmax = outp.tile([P, 1], f32, tag="rmax")
            nc.vector.tensor_reduce(out=rmax[:], in_=mskd[:],
                                    axis=mybir.AxisListType.X, op=Alu.max)
            eq = outp.tile([P, C], f32, tag="eq")
            nc.vector.tensor_scalar(out=eq[:], in0=mskd[:], scalar1=rmax[:],
                                    scalar2=None, op0=Alu.is_ge)
            t1 = outp.tile([P, C], f32, tag="t1")
            nc.vector.tensor_mul(t1[:], eq[:], iotam[:])
            predf = outp.tile([P, 1], f32, tag="predf")
            nc.vector.tensor_reduce(out=predf[:], in_=t1[:],
                                    axis=mybir.AxisListType.X, op=Alu.min)
            predi = outp.tile([P, 1], i32, tag="predi")
            nc.vector.tensor_scalar(out=predi[:], in0=predf[:], scalar1=64.0,
                                    scalar2=None, op0=Alu.add)
            nc.sync.dma_start(prd_d[bass.ts(n, P), :], predi[:])

            # bce = max(s,0) - s*t + softplus(-|s|)
            ab = outp.tile([P, C], f32, tag="ab")
            nc.scalar.activation(ab[:], sc2[:], Act.Abs)
            ex = outp.tile([P, C], f32, tag="ex")
            nc.scalar.activation(ex[:], ab[:], Act.Exp, scale=-1.0)
            sp = outp.tile([P, C], f32, tag="sp")
            nc.scalar.activation(sp[:], ex[:], Act.Ln, bias=1.0)
            st = outp.tile([P, C], f32, tag="st")
            nc.vector.tensor_mul(st[:], sc2[:], tgt[:, n, :])
            rl = outp.tile([P, C], f32, tag="rl")
            nc.vector.scalar_tensor_tensor(out=rl[:], in0=sc2[:], scalar=0.0,
                                           in1=st[:], op0=Alu.max,
                                           op1=Alu.subtract)
            bce = outp.tile([P, C], f32, tag="bce")
            nc.vector.tensor_add(bce[:], rl[:], sp[:])
            mb = outp.tile([P, C], f32, tag="mb")
            nc.vector.scalar_tensor_tensor(
                out=mb[:], in0=bce[:], scalar=0.0, in1=mask[:, n, :],
                op0=Alu.bypass, op1=Alu.mult,
                accum_out=losscol[:, n:n + 1])

        lvec = outp.tile([P, 1], f32, tag="lvec")
        nc.vector.tensor_reduce(out=lvec[:], in_=losscol[:],
                                axis=mybir.AxisListType.X, op=Alu.add)
        nc.sync.dma_start(los_d[:], lvec[:])

    nc.compile()
    return nc


def _prepare(inputs):
    """Host-side input prep; returns (Pt, in_maps)."""
    f32 = np.float32
    sv = np.ascontiguousarray(np.asarray(inputs["span_vecs"], f32))
    ent = np.ascontiguousarray(np.asarray(inputs["ent_table"], f32))
    W1 = np.asarray(inputs["W1"], f32)
    b1 = np.asarray(inputs["b1"], f32)
    W2 = np.asarray(inputs["W2"], f32)
    b2 = np.float32(np.asarray(inputs["b2"], f32))
    tg = np.ascontiguousarray(np.asarray(inputs["targets_all"], f32))
    li = np.ascontiguousarray(np.asarray(inputs["linker_indices"], np.int32))
    ca = np.ascontiguousarray(np.asarray(inputs["candidates_all"], np.int32))
    le = np.ascontiguousarray(
        np.asarray(inputs["candidate_lengths_all"], np.int32))

    # fold W2 into W1/b1 (signed) and permute H: positive-W2 columns first
    pos = W2 >= 0
    perm = np.concatenate([np.nonzero(pos)[0], np.nonzero(~pos)[0]])
    npos = int(pos.sum())
    Pt = [int(np.clip(npos - hc * 512, 0, 512)) for hc in range(HC)]
    Wp = np.ascontiguousarray((W1 * W2[None, :])[:, perm], f32)
    b1p = np.ascontiguousarray((b1 * W2)[perm], f32)

    w1s_in = np.ascontiguousarray(Wp[:D])
    w1e_in = np.ascontiguousarray(Wp[D:])
    b1r_in = np.ascontiguousarray(np.broadcast_to(b1p[None, :], (P, H)), f32)
    ident = np.eye(P, dtype=f32)
    iota = np.ascontiguousarray(
        np.broadcast_to(np.arange(C, dtype=f32)[None, :], (P, C)))
    iotam = np.ascontiguousarray(iota - 64.0)
    b2c = np.full((P, 1), b2, f32)

    in_maps = []
    for b in range(B):
        in_maps.append({
            "span_vecs": sv[b],
            "ent_table": ent,
            "w1s": w1s_in,
            "w1e": w1e_in,
            "b1rep": b1r_in,
            "targets": tg[b],
            "lidx": np.ascontiguousarray(li[b].reshape(SB, P).T),
            "cands": ca[b],
            "lens": le[b].reshape(N, 1),
            "ident": ident,
            "identr": ident,
            "iota": iota,
            "iotam": iotam,
            "b2c": b2c,
        })
    return Pt, in_maps


def kernel(**inputs):
    import sys
    for p in ("/opt/trn_rl_repo", "/root/.axon_site/_ro/trn_rl_repo"):
        if p not in sys.path:
            sys.path.append(p)
    from concourse.bass_utils import run_bass_kernel_spmd

    Pt, in_maps = _prepare(inputs)
    nc = _build_program(Pt)

    trace = bool(int(__import__("os").environ.get("KERNEL_TRACE", "0")))
    res = run_bass_kernel_spmd(nc, in_maps, core_ids=list(range(NCORES)),
                               trace=trace)
    if trace:
        print(f"HW exec time: {res.exec_time_ns} ns")
        if res.instructions_and_trace is not None:
            print("trace:", res.instructions_and_trace[1])

    scores = np.stack([res.results[b]["scores_o"] for b in range(B)])
    preds = np.stack([res.results[b]["preds_o"][:, 0] for b in range(B)])
    loss = np.float32(sum(float(res.results[b]["loss_o"].sum(dtype=np.float64))
                          for b in range(B)))
    return loss, scores, preds
